# revision 34
# baseline (speedup 1.0000x reference)
"""Trainium2 Bass/Tile kernel for nn_Channel_Embedding (moe_routing).

Reference computation (see problem): per-group gating (softmax top-2 over 8
experts) + grouped conv1d (k=3) + tanh + grouped 1x1 expert conv + gate
combine, plus a load-balancing cv^2 loss.

Sharding: data-parallel over batch B=64 across 8 cores (8 samples/core).
Gating for the FULL batch is computed redundantly on every core from a tiny
replicated slice of x (needed for the global load/importance loss); each core
picks out its local 8 gate rows with a per-core one-hot selection matmul.

Key fusion: conv2 (1x1) + expert combine is folded into a per-sample
effective weight  w_eff[b,g,d,c] = sum_e gates[b,g,e] * w2[g,d,e,c]
so the expert dimension never materializes.  conv1 runs as 3 PSUM-accumulated
block-diagonal matmuls (4 groups per 128-row matmul) in fp32r.
"""

import numpy as np
from contextlib import ExitStack

import concourse.bass as bass
import concourse.bacc as bacc
import concourse.mybir as mybir
import concourse.tile as tile
from concourse.bass_utils import run_bass_kernel_spmd

F32 = mybir.dt.float32
F32R = mybir.dt.float32r
ALU = mybir.AluOpType
ACTF = mybir.ActivationFunctionType
AX = mybir.AxisListType

G, DIM, E, OC = 8, 32, 8, 10
B, C, L = 64, 256, 4096
LP = L - 2          # 4094 output length
NCORES = 8
BSH = B // NCORES   # 8 samples per core
LT = 512            # L tile (one PSUM bank of fp32)
NT = (LP + LT - 1) // LT  # 8 tiles, last = 510
HBUFS = 3           # h-tile double/triple buffering

# test-harness knobs (harmless defaults for grading)
TRACE = False
LAST_EXEC_NS = None
LAST_RESULTS = None


def _emit(nc, xs, xgt, w1bd, w2sb, wgbd, c1b, selg, bsel, out, og, ol, reps=1):
    with tile.TileContext(nc) as tc, ExitStack() as ctx:
        singles = ctx.enter_context(tc.tile_pool(name="singles", bufs=1))
        xpool = ctx.enter_context(tc.tile_pool(name="xpool", bufs=6))
        xrpool = ctx.enter_context(tc.tile_pool(name="xrpool", bufs=3))
        hpool = ctx.enter_context(tc.tile_pool(name="hpool", bufs=HBUFS))
        opool = ctx.enter_context(tc.tile_pool(name="opool", bufs=3))
        wbpool = ctx.enter_context(tc.tile_pool(name="wbpool", bufs=2))
        ps1 = ctx.enter_context(tc.tile_pool(name="ps1", bufs=2, space="PSUM"))
        ps2 = ctx.enter_context(tc.tile_pool(name="ps2", bufs=2, space="PSUM"))
        pss = ctx.enter_context(tc.tile_pool(name="pss", bufs=2, space="PSUM"))

        dma = nc.sync.dma_start

        # ---------- constants into SBUF ----------
        # NOTE: this walrus build only allows ONE sync-wait on a (fused-LDW)
        # matmul instruction, so every matmul input must be last-written by a
        # single engine.  DMA-landed tiles are therefore funneled through DVE
        # (or ACT) copies before the PE reads them.
        w1f = singles.tile([128, 2, 3, 40], F32, tag="w1f")
        dma(out=w1f, in_=w1bd[:])
        w1 = singles.tile([128, 2, 3, 40], F32R, tag="w1")
        nc.vector.tensor_copy(w1, w1f)          # f32 -> f32r round
        w2 = singles.tile([64, 110], F32, tag="w2")
        dma(out=w2, in_=w2sb[:])
        wgf = singles.tile([128, 2, 5, 32], F32, tag="wgf")
        dma(out=wgf, in_=wgbd[:])
        wg = singles.tile([128, 2, 5, 32], F32, tag="wg")
        nc.vector.tensor_copy(wg, wgf)
        b1t = singles.tile([40, 2], F32, tag="b1t")
        dma(out=b1t, in_=c1b[:])
        self_sel = singles.tile([64, 8], F32, tag="self_sel")
        dma(out=self_sel, in_=selg[:])
        sel = singles.tile([64, 8], F32, tag="sel")
        nc.vector.tensor_copy(sel, self_sel)
        bsf = singles.tile([64, 8], F32, tag="bsf")
        dma(out=bsf, in_=bsel[:])
        bs = singles.tile([64, 8], F32, tag="bs")
        nc.vector.tensor_copy(bs, bsf)
        xgf = singles.tile([128, 2, 5, 64], F32, tag="xgf")
        dma(out=xgf, in_=xgt.rearrange("(gh c) j b -> c gh j b", gh=2))
        xg = singles.tile([128, 2, 5, 64], F32, tag="xg")
        nc.vector.tensor_copy(xg, xgf)
        ones_col = singles.tile([64, 1], F32, tag="ones")
        nc.vector.memset(ones_col, 1.0)

        # ---------- gating: logits for the FULL batch ----------
        # pg[b, g*8+e] = sum_{d,j} x[b, g*32+d, L-6+j] * w_gate[g, d*5+j, e]
        pg = pss.tile([64, 64], F32, tag="pssm")
        for gh in range(2):
            for j in range(5):
                nc.tensor.matmul(
                    pg[:, gh * 32:(gh + 1) * 32],
                    lhsT=xg[:, gh, j, :],
                    rhs=wg[:, gh, j, :],
                    start=(j == 0), stop=(j == 4),
                )

        # softmax (no max-shift; logits are small) + top-2 normalize.
        ep = singles.tile([64, 8, 8], F32, tag="ep")
        nc.scalar.activation(ep, pg.rearrange("p (g e) -> p g e", g=8), ACTF.Exp)
        sums = singles.tile([64, 8], F32, tag="sums")
        nc.vector.reduce_sum(out=sums, in_=ep, axis=AX.X)
        m1 = singles.tile([64, 8], F32, tag="m1")
        nc.vector.reduce_max(out=m1, in_=ep, axis=AX.X)
        msk1 = singles.tile([64, 8, 8], F32, tag="msk1")
        nc.vector.tensor_tensor(msk1, ep, m1.unsqueeze(2).to_broadcast([64, 8, 8]), ALU.is_lt)
        ep2 = singles.tile([64, 8, 8], F32, tag="ep2")
        nc.vector.tensor_tensor(ep2, ep, msk1, ALU.mult)
        m2 = singles.tile([64, 8], F32, tag="m2")
        nc.vector.reduce_max(out=m2, in_=ep2, axis=AX.X)
        # denom = m1 + m2 + 1e-6 * sum  (all in un-normalized exp space)
        den = singles.tile([64, 8], F32, tag="den")
        nc.vector.tensor_tensor(den, m1, m2, ALU.add)
        den2 = singles.tile([64, 8], F32, tag="den2")
        nc.vector.scalar_tensor_tensor(out=den2, in0=sums, scalar=1e-6, in1=den,
                                       op0=ALU.mult, op1=ALU.add)
        rden = singles.tile([64, 8], F32, tag="rden")
        nc.vector.reciprocal(rden, den2)
        mask = singles.tile([64, 8, 8], F32, tag="mask")  # top-2 indicator
        nc.vector.tensor_tensor(mask, ep, m2.unsqueeze(2).to_broadcast([64, 8, 8]), ALU.is_ge)
        gtv = singles.tile([64, 8, 8], F32, tag="gtv")
        nc.vector.tensor_tensor(gtv, ep, mask, ALU.mult)
        gates = singles.tile([64, 8, 8], F32, tag="gates")
        nc.vector.tensor_tensor(gates, gtv, rden.unsqueeze(2).to_broadcast([64, 8, 8]), ALU.mult)
        gates_f = gates.rearrange("p g e -> p (g e)")
        mask_f = mask.rearrange("p g e -> p (g e)")
        dma(out=og[:], in_=gates_f)

        # ---------- loss: cv^2 of importance & load ----------
        pimp = pss.tile([1, 64], F32, tag="pssm")
        nc.tensor.matmul(pimp, lhsT=ones_col, rhs=gates_f, start=True, stop=True)
        pload = pss.tile([1, 64], F32, tag="pssm")
        nc.tensor.matmul(pload, lhsT=ones_col, rhs=mask_f, start=True, stop=True)

        def cv_sum(pv, tp):
            # pv: [1, 64] psum; returns [1,1] sbuf = sum_g (n*var)/(mean^2+eps)
            # where num = sum(v^2) - (sum v)^2/8  (later scaled by 1/7)
            vv = singles.tile([1, 64], F32, tag=tp + "vv")
            nc.vector.tensor_copy(vv, pv)
            v = vv.rearrange("p (g e) -> p g e", g=8)
            s1 = singles.tile([1, 8], F32, tag=tp + "s1")
            nc.vector.reduce_sum(out=s1, in_=v, axis=AX.X)
            sq = singles.tile([1, 8, 8], F32, tag=tp + "sq")
            nc.vector.tensor_tensor(sq, v, v, ALU.mult)
            s2 = singles.tile([1, 8], F32, tag=tp + "s2")
            nc.vector.reduce_sum(out=s2, in_=sq, axis=AX.X)
            a = singles.tile([1, 8], F32, tag=tp + "a")
            nc.vector.tensor_tensor(a, s1, s1, ALU.mult)
            num = singles.tile([1, 8], F32, tag=tp + "num")
            nc.vector.scalar_tensor_tensor(out=num, in0=a, scalar=-0.125, in1=s2,
                                           op0=ALU.mult, op1=ALU.add)
            dd = singles.tile([1, 8], F32, tag=tp + "dd")
            nc.vector.tensor_scalar(out=dd, in0=a, scalar1=1.0 / 64.0, scalar2=1e-10,
                                    op0=ALU.mult, op1=ALU.add)
            r = singles.tile([1, 8], F32, tag=tp + "r")
            nc.vector.reciprocal(r, dd)
            cv = singles.tile([1, 8], F32, tag=tp + "cv")
            nc.vector.tensor_tensor(cv, num, r, ALU.mult)
            s = singles.tile([1, 1], F32, tag=tp + "s")
            nc.vector.reduce_sum(out=s, in_=cv, axis=AX.X)
            return s

        simp = cv_sum(pimp, "i")
        sload = cv_sum(pload, "l")
        tot = singles.tile([1, 1], F32, tag="tot")
        nc.vector.tensor_tensor(tot, simp, sload, ALU.add)
        lossv = singles.tile([1, 1], F32, tag="lossv")
        nc.scalar.mul(lossv, tot, 0.01 / 7.0)
        dma(out=ol[:], in_=lossv)

        # ---------- local gate columns: glT[(g,e), b_local] ----------
        pgl = pss.tile([64, 8], F32, tag="pssm")
        nc.tensor.matmul(pgl, lhsT=gates_f, rhs=bs, start=True, stop=True)
        gl = singles.tile([64, 8], F32, tag="gl")
        nc.scalar.copy(gl, pgl)

        # ---------- per-sample effective conv2 weights (hoisted) ----------
        # pw[g, c*10+d] = sum_e gates[b,g,e] * w2[g,d,e,c];  pw[g,100+d] = b_eff
        # All 8 samples' weights are packed into ONE persistent lhsT tile
        # w2ball[40, hh, b, 80] so the block-diagonal scatter needs only 8
        # DMAs total (HWDGE descriptor generation is ~0.6us each, serialized).
        pwsall = singles.tile([8, 8 * 110], F32, tag="pwsall")
        for bl in range(BSH):
            tmp = wbpool.tile([64, 110], F32, tag="tmp")
            nc.vector.tensor_scalar(out=tmp, in0=w2, scalar1=gl[:, bl:bl + 1],
                                    scalar2=None, op0=ALU.mult)
            pw = pss.tile([8, 110], F32, tag="pssm")
            nc.tensor.matmul(pw, lhsT=sel, rhs=tmp, start=True, stop=True)
            nc.vector.tensor_copy(pwsall[:, bl * 110:(bl + 1) * 110], pw)
        # scatter the per-sample blocks into per-sample f32r lhsT tiles
        # (separate tiles keep conv2(b) dependent on only ITS 8 writers, so
        # the first samples' conv2 need not wait for the whole scatter pass).
        # SWDGE (gpsimd, casting) DMAs keep the serialized HWDGE generator
        # free for the big input/output transfers.
        pwsv = pwsall.rearrange("p (b c d) -> p b c d", b=8, c=11)
        w2bb = []
        for bl in range(BSH):
            t_ = singles.tile([40, 2, 80], F32R, tag=f"w2bb{bl}", name=f"w2bb{bl}")
            w2bb.append(t_)
            nc.vector.memzero(t_)
            for g in range(G):
                hh, glo = divmod(g, 4)
                nc.gpsimd.dma_start(
                    out=t_[glo * 10:(glo + 1) * 10, hh, g * 10:(g + 1) * 10],
                    in_=pwsv[g:g + 1, bl, 0:10, :])
        beffall = singles.tile([80, 8], F32, tag="beffall")
        for bl in range(BSH):
            dma(out=beffall[:, bl:bl + 1],
                in_=pwsv[:, bl, 10:11, :])   # bias col block: [g, 1, d]

        # ---------- main per-sample pipeline ----------
        for bl_rep in range(BSH * reps):
            bl = bl_rep % BSH
            ot = opool.tile([80, LP], F32, tag="ot")
            for t in range(NT):
                l0 = t * LT
                n = min(LT, LP - l0)
                nn = min(n + 2, L - l0)
                xbt = xpool.tile([128, 2, LT + 2], F32R, tag="xbt")
                dma(out=xbt[:, :, :nn],
                    in_=xs[bl].rearrange("(gh c) l -> c gh l", gh=2)[:, :, l0:l0 + nn])
                p1 = [ps1.tile([40, LT], F32, tag=f"p1{hh}", name=f"p1{hh}") for hh in range(2)]
                for gh in range(2):
                    for k in range(3):
                        nc.tensor.matmul(
                            p1[gh][:, :n],
                            lhsT=w1[:, gh, k, :],
                            rhs=xbt[:, gh, k:k + n],
                            start=(k == 0), stop=(k == 2),
                        )
                hh_t = [hpool.tile([40, LT], F32R, tag=f"h{hh}", name=f"h{hh}") for hh in range(2)]
                for gh in range(2):
                    nc.scalar.activation(hh_t[gh][:, :n], p1[gh][:, :n],
                                         ACTF.Tanh, bias=b1t[:, gh:gh + 1])
                p2 = ps2.tile([80, LT], F32, tag="p2")
                nc.tensor.matmul(p2[:, :n], lhsT=w2bb[bl][:, 0, :], rhs=hh_t[0][:, :n],
                                 start=True, stop=False)
                nc.tensor.matmul(p2[:, :n], lhsT=w2bb[bl][:, 1, :], rhs=hh_t[1][:, :n],
                                 start=False, stop=True)
                nc.vector.tensor_scalar(out=ot[:, l0:l0 + n], in0=p2[:, :n],
                                        scalar1=beffall[:, bl:bl + 1],
                                        scalar2=None, op0=ALU.add)
            dma(out=out[bl], in_=ot)
    return nc


def build_module(reps=1):
    nc = bacc.Bacc("TRN2", target_bir_lowering=False, debug=False)
    xs = nc.declare_dram_parameter("xs", [BSH, C, L], F32R, isOutput=False)
    xgt = nc.declare_dram_parameter("xgt", [C, 5, B], F32, isOutput=False)
    w1bd = nc.declare_dram_parameter("w1bd", [128, 2, 3, 40], F32, isOutput=False)
    w2sb = nc.declare_dram_parameter("w2sb", [64, 110], F32, isOutput=False)
    wgbd = nc.declare_dram_parameter("wgbd", [128, 2, 5, 32], F32, isOutput=False)
    c1b = nc.declare_dram_parameter("c1b", [40, 2], F32, isOutput=False)
    selg = nc.declare_dram_parameter("selg", [64, 8], F32, isOutput=False)
    bsel = nc.declare_dram_parameter("bsel", [64, 8], F32, isOutput=False)
    out = nc.declare_dram_parameter("out", [BSH, 80, LP], F32, isOutput=True)
    og = nc.declare_dram_parameter("og", [64, 64], F32, isOutput=True)
    ol = nc.declare_dram_parameter("ol", [1, 1], F32, isOutput=True)
    _emit(nc, xs, xgt, w1bd, w2sb, wgbd, c1b, selg, bsel, out, og, ol, reps=reps)
    nc.compile()
    return nc


def pack_inputs(x, conv1_w, conv1_b, conv2_w, conv2_b, w_gate):
    """Host-side layout packing of the (tiny) weights + gating slice."""
    x = np.ascontiguousarray(x, dtype=np.float32)
    # gating slice, transposed for K-on-partitions matmul: [C, 5, B]
    xgt = np.ascontiguousarray(x[:, :, L - 6:L - 1].transpose(1, 2, 0))
    # conv1 block-diagonal lhsT per (half, tap): [128, 2, 3, 40]
    w1bd = np.zeros((128, 2, 3, 40), np.float32)
    w1r = conv1_w.reshape(2, 4, OC, DIM, 3)  # [gh, gl, oc, d, k]
    for gh in range(2):
        for gl in range(4):
            # dest[d, k, oc] block at rows gl*32, cols gl*10
            w1bd[gl * 32:gl * 32 + 32, gh, :, gl * 10:gl * 10 + 10] = \
                w1r[gh, gl].transpose(1, 2, 0)  # [oc,d,k] -> [d, k, oc]
    # conv2 weights as rhs [64, 110]: rows (g,e); cols c*10+d, then bias cols 100+d
    w2r = conv2_w[:, :, 0].reshape(G, OC, E, OC)     # [g, d, e, c]
    w2sb = np.zeros((64, 110), np.float32)
    w2sb[:, :100] = w2r.transpose(0, 2, 3, 1).reshape(64, 100)  # [g,e][c,d]
    w2sb[:, 100:] = conv2_b.reshape(G, OC, E).transpose(0, 2, 1).reshape(64, OC)
    # gate weights block-diagonal: [128, 2, 5, 32]
    wgbd = np.zeros((128, 2, 5, 32), np.float32)
    wgr = w_gate.reshape(2, 4, DIM, 5, E)  # [gh, gl, d, j, e]
    for gh in range(2):
        for gl in range(4):
            wgbd[gl * 32:gl * 32 + 32, gh, :, gl * 8:gl * 8 + 8] = wgr[gh, gl]  # [d, j, e]
    selg = np.zeros((64, 8), np.float32)
    for g in range(G):
        selg[g * 8:(g + 1) * 8, g] = 1.0
    return x, xgt, w1bd, w2sb, wgbd, selg


def kernel(x, conv1_w, conv1_b, conv2_w, conv2_b, w_gate):
    x = np.asarray(x, np.float32)
    conv1_w = np.asarray(conv1_w, np.float32)
    conv1_b = np.asarray(conv1_b, np.float32)
    conv2_w = np.asarray(conv2_w, np.float32)
    conv2_b = np.asarray(conv2_b, np.float32)
    w_gate = np.asarray(w_gate, np.float32)

    x, xgt, w1bd, w2sb, wgbd, selg = pack_inputs(
        x, conv1_w, conv1_b, conv2_w, conv2_b, w_gate)

    nc = build_module()
    in_maps = []
    for c in range(NCORES):
        bsel = np.zeros((64, 8), np.float32)
        for i in range(BSH):
            bsel[c * BSH + i, i] = 1.0
        in_maps.append({
            "xs": np.ascontiguousarray(x[c * BSH:(c + 1) * BSH]),
            "xgt": xgt, "w1bd": w1bd, "w2sb": w2sb, "wgbd": wgbd,
            "c1b": np.ascontiguousarray(conv1_b.reshape(2, 40).T), "selg": selg, "bsel": bsel,
        })
    try:
        res = run_bass_kernel_spmd(nc, in_maps, core_ids=list(range(NCORES)),
                                   trace=TRACE)
    except ModuleNotFoundError:
        # axon NTFF profiling hook unavailable in this container
        res = run_bass_kernel_spmd(nc, in_maps, core_ids=list(range(NCORES)),
                                   trace=False)
    global LAST_EXEC_NS, LAST_RESULTS
    LAST_EXEC_NS = res.exec_time_ns
    LAST_RESULTS = res
    outs = [res.results[c]["out"] for c in range(NCORES)]
    combine = np.concatenate(outs, axis=0).reshape(B, G, OC, LP)
    loss = np.float32(res.results[0]["ol"][0, 0])
    gates = res.results[0]["og"].reshape(B, G, E).transpose(0, 2, 1)
    return combine, loss, np.ascontiguousarray(gates)


def bench_device(n_iters=30, reps=1):
    """Time repeated device executions of the same program (no H2D in loop).

    Mirrors bass2jax.run_bass_via_pjrt's shard_map path, but jits once,
    device_puts the sharded inputs once, and times back-to-back runs.
    Returns (mean_ns_pipelined, min_ns_single, results_from_last_run).
    """
    import time
    import jax
    from jax.sharding import Mesh, PartitionSpec, NamedSharding
    from jax.experimental.shard_map import shard_map
    import concourse.bass2jax as b2j
    import concourse.mybir as mybir_

    nc, in_maps = _pack_from_setup(reps=reps)

    b2j.install_neuronx_cc_hook()
    in_names, out_names, out_avals, zero_outs = [], [], [], []
    for alloc in nc.m.functions[0].allocations:
        if not isinstance(alloc, mybir_.MemoryLocationSet):
            continue
        name = alloc.memorylocations[0].name
        partition_name = (nc.partition_id_tensor.name
                          if nc.partition_id_tensor else None)
        if alloc.kind == "ExternalInput":
            if name != partition_name:
                in_names.append(name)
        elif alloc.kind == "ExternalOutput":
            out_names.append(name)
            shape = tuple(alloc.tensor_shape)
            dtype = mybir_.dt.np(alloc.dtype)
            out_avals.append(jax.core.ShapedArray(shape, dtype))
            zero_outs.append(np.zeros(shape, dtype))
    n_params = len(in_names)
    all_in_names = in_names + out_names
    if nc.partition_id_tensor is not None:
        all_in_names = all_in_names + [nc.partition_id_tensor.name]

    def _body(*args):
        operands = list(args)
        if nc.partition_id_tensor is not None:
            operands.append(b2j.partition_id_tensor())
        outs = b2j._bass_exec_p.bind(
            *operands,
            out_avals=tuple(out_avals),
            in_names=tuple(all_in_names),
            out_names=tuple(out_names),
            lowering_input_output_aliases=(),
            sim_require_finite=True,
            sim_require_nnan=True,
            nc=nc,
        )
        return tuple(outs)

    devices = jax.devices()[:NCORES]
    mesh = Mesh(np.asarray(devices), ("core",))
    spec = PartitionSpec("core")
    sharded = jax.jit(
        shard_map(_body, mesh=mesh,
                  in_specs=(spec,) * (n_params + len(out_names)),
                  out_specs=(spec,) * len(out_names), check_rep=False),
        keep_unused=True,
    )
    concat_in = [
        np.concatenate([np.asarray(in_maps[c][nm])[None] for c in range(NCORES)], axis=0
                       ).reshape(NCORES * in_maps[0][nm].shape[0] if False else -1, *in_maps[0][nm].shape[1:])
        for nm in in_names
    ]
    # proper concat along axis 0 (shard dim)
    concat_in = [
        np.concatenate([np.asarray(in_maps[c][nm]) for c in range(NCORES)], axis=0)
        for nm in in_names
    ]
    concat_zero = [np.zeros((NCORES * z.shape[0], *z.shape[1:]), z.dtype) for z in zero_outs]
    sh = NamedSharding(mesh, spec)
    dev_in = [jax.device_put(a, sh) for a in concat_in + concat_zero]
    # warm-up / compile
    outs = sharded(*dev_in)
    jax.block_until_ready(outs)
    # single-call min
    best = float("inf")
    for _ in range(5):
        t0 = time.perf_counter()
        outs = sharded(*dev_in)
        jax.block_until_ready(outs)
        best = min(best, time.perf_counter() - t0)
    # pipelined average — vary one small input per call so any
    # content-addressed execute caching on the axon terminal misses
    gi = in_names.index("selg")
    variants = []
    for it in range(n_iters):
        sv = np.concatenate([np.asarray(in_maps[c]["selg"]) for c in range(NCORES)], axis=0).copy()
        sv[0, 0] += 1e-6 * (it + 1)
        variants.append(jax.device_put(sv, sh))
    jax.block_until_ready(variants)
    t0 = time.perf_counter()
    rs = []
    for it in range(n_iters):
        din2 = list(dev_in)
        din2[gi] = variants[it]
        rs.append(sharded(*din2))
    jax.block_until_ready(rs)
    pipelined = (time.perf_counter() - t0) / n_iters
    return pipelined * 1e9, best * 1e9, outs


def _pack_from_setup(reps=1):
    """Build (nc, in_maps) from reference-distribution inputs (test helper)."""
    rng = np.random.RandomState(0)
    x = rng.randn(B, C, L).astype(np.float32)
    conv1_w = (rng.randn(80, 32, 3) * 0.05).astype(np.float32)
    conv1_b = (rng.randn(80) * 0.05).astype(np.float32)
    conv2_w = (rng.randn(640, 10, 1) * 0.1).astype(np.float32)
    conv2_b = (rng.randn(640) * 0.05).astype(np.float32)
    w_gate = (rng.randn(8, 160, 8) * 0.05).astype(np.float32)
    x, xgt, w1bd, w2sb, wgbd, selg = pack_inputs(
        x, conv1_w, conv1_b, conv2_w, conv2_b, w_gate)
    nc = build_module(reps=reps)
    in_maps = []
    for c in range(NCORES):
        bsel = np.zeros((64, 8), np.float32)
        for i in range(BSH):
            bsel[c * BSH + i, i] = 1.0
        in_maps.append({
            "xs": np.ascontiguousarray(x[c * BSH:(c + 1) * BSH]),
            "xgt": xgt, "w1bd": w1bd, "w2sb": w2sb, "wgbd": wgbd,
            "c1b": np.ascontiguousarray(conv1_b.reshape(2, 40).T),
            "selg": selg, "bsel": bsel,
        })
    return nc, in_maps
